# revision 3
# baseline (speedup 1.0000x reference)
"""Multi-head bilinear attention on 8 Trainium2 NeuronCores.

Math (per batch b, head h):
    scores = (X @ M_h @ X^T) / 64        [S, S]
    A      = softmax(scores, axis=-1)
    V      = X @ Wv_h                    [S, Q]
    Zh     = A @ V                       [S, Q]
    out    = concat_h(Zh) @ W0           [S, Z]

Sharding: core c handles batch b = c // 4 and heads (2*(c%4), 2*(c%4)+1).
Each core computes its two heads' attention plus their partial W0
projection; the host sums the 4 partials per batch.

Device kernel (per core, all bf16 matmuls, fp32 accumulation):
  phase 1 (per head): XMT[m, s] = (X @ M_h)^T, V_aug = [X @ Wv_h | 1]
  phase 2 (per head, per 512-wide s-block):
      for each 128-row t-chunk: scoresT[t, s] via PE, exp via ACT,
      AV accumulation via PE with lhsT = V_aug (row 64 accumulates the
      softmax denominator).  Normalization: reciprocal of the denominator
      row, broadcast across partitions with a K=1 matmul, multiplied in
      while copying ZhT out of PSUM.
  phase 3: out[s, z] = sum_h ZhT_h^T @ W0_h, accumulated in PSUM.

The softmax skips max-subtraction: scores/64 is bounded by ~±2 for these
glorot/randn inputs, so exp() cannot overflow and softmax(x) == softmax(x-m).
"""

import numpy as np
import ml_dtypes

B, S, N, Q, H, Z = 2, 4096, 256, 64, 8, 512
NCORES = 8
HEADS_PER_CORE = 2
SB = 512          # s-block width (PSUM bank width in fp32)
NSB = S // SB     # 8 s-blocks
NT = S // 128     # 32 t-chunks
TB = 3            # t-chunks per exp batch (PSUM: 2*3 + 1 + 1 = 8 banks)

_bf16 = ml_dtypes.bfloat16


def _build_program():
    import concourse.mybir as mybir
    import concourse.tile as tile
    from concourse import bacc

    f32 = mybir.dt.float32
    bf16 = mybir.dt.bfloat16
    Exp = mybir.ActivationFunctionType.Exp

    nc = bacc.Bacc(None, target_bir_lowering=False)
    xt_d = nc.declare_dram_parameter("xt", [N, S], bf16, isOutput=False)
    m_d = nc.declare_dram_parameter("m", [HEADS_PER_CORE, N, N], bf16, isOutput=False)
    wv_d = nc.declare_dram_parameter("wv", [HEADS_PER_CORE, N, Q], bf16, isOutput=False)
    w0_d = nc.declare_dram_parameter("w0", [HEADS_PER_CORE, Q, Z], bf16, isOutput=False)
    out_d = nc.declare_dram_parameter("out", [S, Z], f32, isOutput=True)

    with tile.TileContext(nc) as tc:
        with (
            tc.tile_pool(name="io", bufs=1) as iop,
            tc.tile_pool(name="work", bufs=1) as workp,
            tc.tile_pool(name="psum", bufs=1, space="PSUM") as psump,
        ):
            # ---- persistent SBUF tensors ----
            xt = iop.tile([128, 2, S], bf16)      # X^T, feature chunks on axis 1
            m_sb = iop.tile([128, HEADS_PER_CORE, 2, N], bf16)
            wv_sb = iop.tile([128, HEADS_PER_CORE, 2, Q], bf16)
            w0_sb = iop.tile([Q, HEADS_PER_CORE, Z], bf16)
            ones_sb = iop.tile([1, 64], f32)
            zht = iop.tile([Q, HEADS_PER_CORE, S], bf16)   # normalized ZhT

            for c in range(2):
                nc.sync.dma_start(xt[:, c, :], xt_d[c * 128:(c + 1) * 128, :])
            for hh in range(HEADS_PER_CORE):
                for nn in range(2):
                    nc.sync.dma_start(
                        m_sb[:, hh, nn, :], m_d[hh, nn * 128:(nn + 1) * 128, :])
                    nc.sync.dma_start(
                        wv_sb[:, hh, nn, :], wv_d[hh, nn * 128:(nn + 1) * 128, :])
                nc.sync.dma_start(w0_sb[:, hh, :], w0_d[hh])
            nc.vector.memset(ones_sb[:], 1.0)

            for hh in range(HEADS_PER_CORE):
                # ---- phase 1: XMT = (X @ M_h)^T as [m, s], bf16 ----
                xmt = workp.tile([128, 2, S], bf16, tag="xmt", bufs=2)
                for mc in range(2):
                    for st in range(NSB):
                        pmm = psump.tile([128, SB], f32, tag="mm", bufs=1)
                        for nn in range(2):
                            nc.tensor.matmul(
                                pmm[:],
                                m_sb[:, hh, nn, mc * 128:(mc + 1) * 128],
                                xt[:, nn, st * SB:(st + 1) * SB],
                                start=(nn == 0), stop=(nn == 1),
                            )
                        nc.vector.tensor_copy(xmt[:, mc, st * SB:(st + 1) * SB], pmm[:])

                # ---- phase 1b: V_aug[t, 0:64] = X @ Wv_h, col 64 = 1.0 ----
                vaug = workp.tile([128, NT, Q + 1], bf16, tag="vaug", bufs=2)
                nc.vector.memset(vaug[:, :, Q:Q + 1], 1.0)
                for t in range(NT):
                    pv = psump.tile([128, Q], f32, tag="mm", bufs=1)
                    for nn in range(2):
                        nc.tensor.matmul(
                            pv[:],
                            xt[:, nn, t * 128:(t + 1) * 128],
                            wv_sb[:, hh, nn, :],
                            start=(nn == 0), stop=(nn == 1),
                        )
                    nc.vector.tensor_copy(vaug[:, t, 0:Q], pv[:])

                # ---- phase 2: attention per s-block ----
                for sb in range(NSB):
                    s_lo = sb * SB
                    pav = psump.tile([Q + 1, SB], f32, tag="av", bufs=1)
                    for j in range((NT + TB - 1) // TB):
                        t0 = j * TB
                        nb = min(TB, NT - t0)
                        pbig = psump.tile([128, TB, SB], f32, tag="big", bufs=2)
                        for kk in range(nb):
                            t = t0 + kk
                            for mc in range(2):
                                nc.tensor.matmul(
                                    pbig[:, kk, :],
                                    xt[:, mc, t * 128:(t + 1) * 128],
                                    xmt[:, mc, s_lo:s_lo + SB],
                                    start=(mc == 0), stop=(mc == 1),
                                )
                        et = workp.tile([128, TB, SB], bf16, tag="et", bufs=3)
                        nc.scalar.activation(et[:, :nb, :], pbig[:, :nb, :], Exp)
                        for kk in range(nb):
                            t = t0 + kk
                            nc.tensor.matmul(
                                pav[:],
                                vaug[:, t, :],
                                et[:, kk, :],
                                start=(t == 0), stop=(t == NT - 1),
                            )
                    # normalize: zht = pav[0:64] * broadcast(1 / pav[64])
                    rs = workp.tile([1, SB], f32, tag="rs", bufs=2)
                    nc.vector.reciprocal(rs[:], pav[Q:Q + 1, :])
                    prb = psump.tile([Q, SB], f32, tag="mm", bufs=1)
                    nc.tensor.matmul(prb[:], ones_sb[:], rs[:])
                    zraw = workp.tile([Q, SB], f32, tag="zraw", bufs=2)
                    nc.vector.tensor_copy(zraw[:], pav[0:Q, :])
                    nc.vector.tensor_mul(zht[:, hh, s_lo:s_lo + SB], zraw[:], prb[:])

            # ---- phase 3: out = sum_h ZhT_h^T @ W0_h ----
            for sc in range(NT):
                po = psump.tile([128, Z], f32, tag="mm", bufs=1)
                for hh in range(HEADS_PER_CORE):
                    nc.tensor.matmul(
                        po[:],
                        zht[:, hh, sc * 128:(sc + 1) * 128],
                        w0_sb[:, hh, :],
                        start=(hh == 0), stop=(hh == HEADS_PER_CORE - 1),
                    )
                ot = workp.tile([128, Z], f32, tag="ot", bufs=3)
                nc.vector.tensor_copy(ot[:], po[:])
                nc.sync.dma_start(out_d[sc * 128:(sc + 1) * 128, :], ot[:])

    nc.compile()
    return nc


_NC_CACHE = None


def _get_program():
    global _NC_CACHE
    if _NC_CACHE is None:
        _NC_CACHE = _build_program()
    return _NC_CACHE


def _make_in_maps(X, M, W_v, W0):
    w0h = np.ascontiguousarray(W0.reshape(H, Q, Z))
    in_maps = []
    for c in range(NCORES):
        b = c // 4
        h0 = HEADS_PER_CORE * (c % 4)
        hs = [h0 + i for i in range(HEADS_PER_CORE)]
        in_maps.append({
            "xt": np.ascontiguousarray(X[b].T).astype(_bf16),
            "m": np.ascontiguousarray(M[hs] / float(Q)).astype(_bf16),
            "wv": np.ascontiguousarray(W_v[hs]).astype(_bf16),
            "w0": w0h[hs].astype(_bf16),
        })
    return in_maps


def run(X, M, W_v, W0, trace=False):
    from concourse.bass_utils import run_bass_kernel_spmd

    nc = _get_program()
    in_maps = _make_in_maps(X, M, W_v, W0)
    res = run_bass_kernel_spmd(nc, in_maps, list(range(NCORES)), trace=trace)
    out = np.zeros((B, S, Z), np.float32)
    for c in range(NCORES):
        out[c // 4] += np.asarray(res.results[c]["out"], dtype=np.float32)
    return out, res


def kernel(X, M, W_v, W0):
    out, _ = run(np.asarray(X), np.asarray(M), np.asarray(W_v), np.asarray(W0))
    return out


# revision 7
# speedup vs baseline: 130.2270x; 130.2270x over previous
"""Multi-head bilinear attention on 8 Trainium2 NeuronCores.

Math (per batch b, head h):
    scores = (X @ M_h @ X^T) / 64        [S, S]
    A      = softmax(scores, axis=-1)
    V      = X @ Wv_h                    [S, Q]
    Zh     = A @ V                       [S, Q]
    out    = concat_h(Zh) @ W0           [S, Z]

Sharding: core c handles batch b = c // 4 and heads (2*(c%4), 2*(c%4)+1).
Each core computes its two heads' attention plus their partial W0
projection; the host sums the 4 partials per batch.

Device kernel (per core, all bf16 matmuls, fp32 accumulation):
  phase 1 (per head): XMT[m, s] = (X @ M_h)^T, V_aug = [X @ Wv_h | 1]
  phase 2 (per head, per 512-wide s-block):
      for each 128-row t-chunk: scoresT[t, s] via PE, exp via ACT,
      AV accumulation via PE with lhsT = V_aug (row 64 accumulates the
      softmax denominator).  Normalization: reciprocal of the denominator
      row, broadcast across partitions with a K=1 matmul, multiplied in
      while copying ZhT out of PSUM.
  phase 3: out[s, z] = sum_h ZhT_h^T @ W0_h, accumulated in PSUM.

The softmax skips max-subtraction: scores/64 is bounded by ~±2 for these
glorot/randn inputs, so exp() cannot overflow and softmax(x) == softmax(x-m).
"""

import numpy as np
import ml_dtypes

B, S, N, Q, H, Z = 2, 4096, 256, 64, 8, 512
NCORES = 8
HEADS_PER_CORE = 2
SB = 512          # s-block width (PSUM bank width in fp32)
NSB = S // SB     # 8 s-blocks
NT = S // 128     # 32 t-chunks
TB = 3            # t-chunks per exp batch (PSUM: 2*3 + 1 + 1 = 8 banks)

_bf16 = ml_dtypes.bfloat16


def _build_program(reps=1):
    import concourse.mybir as mybir
    import concourse.tile as tile
    from concourse import bacc

    f32 = mybir.dt.float32
    bf16 = mybir.dt.bfloat16
    Exp = mybir.ActivationFunctionType.Exp

    nc = bacc.Bacc(None, target_bir_lowering=False)
    xt_d = nc.declare_dram_parameter("xt", [N, S], bf16, isOutput=False)
    m_d = nc.declare_dram_parameter("m", [HEADS_PER_CORE, N, N], bf16, isOutput=False)
    wv_d = nc.declare_dram_parameter("wv", [HEADS_PER_CORE, N, Q], bf16, isOutput=False)
    w0_d = nc.declare_dram_parameter("w0", [HEADS_PER_CORE, Q, Z], bf16, isOutput=False)
    out_d = nc.declare_dram_parameter("out", [S, Z], f32, isOutput=True)

    with tile.TileContext(nc) as tc:
        with (
            tc.tile_pool(name="io", bufs=1) as iop,
            tc.tile_pool(name="work", bufs=1) as workp,
            tc.tile_pool(name="psum", bufs=1, space="PSUM") as psump,
        ):
            # ---- persistent SBUF tensors ----
            xt = iop.tile([128, 2, S], bf16)      # X^T, feature chunks on axis 1
            m_sb = iop.tile([128, HEADS_PER_CORE, 2, N], bf16)
            wv_sb = iop.tile([128, HEADS_PER_CORE, 2, Q], bf16)
            w0_sb = iop.tile([Q, HEADS_PER_CORE, Z], bf16)
            ones_sb = iop.tile([1, 64], f32)
            zht = iop.tile([Q, HEADS_PER_CORE, S], bf16)   # normalized ZhT

            for c in range(2):
                nc.sync.dma_start(xt[:, c, :], xt_d[c * 128:(c + 1) * 128, :])
            for hh in range(HEADS_PER_CORE):
                for nn in range(2):
                    nc.sync.dma_start(
                        m_sb[:, hh, nn, :], m_d[hh, nn * 128:(nn + 1) * 128, :])
                    nc.sync.dma_start(
                        wv_sb[:, hh, nn, :], wv_d[hh, nn * 128:(nn + 1) * 128, :])
                nc.sync.dma_start(w0_sb[:, hh, :], w0_d[hh])
            nc.vector.memset(ones_sb[:], 1.0)

            for _rep in range(reps):
              for hh in range(HEADS_PER_CORE):
                # ---- phase 1: XMT = (X @ M_h)^T as [m, s], bf16 ----
                xmt = workp.tile([128, 2, S], bf16, tag="xmt", bufs=2)
                for mc in range(2):
                    for st in range(NSB):
                        pmm = psump.tile([128, SB], f32, tag="mm", bufs=1)
                        for nn in range(2):
                            nc.tensor.matmul(
                                pmm[:],
                                m_sb[:, hh, nn, mc * 128:(mc + 1) * 128],
                                xt[:, nn, st * SB:(st + 1) * SB],
                                start=(nn == 0), stop=(nn == 1),
                            )
                        nc.vector.tensor_copy(xmt[:, mc, st * SB:(st + 1) * SB], pmm[:])

                # ---- phase 1b: V_aug[t, 0:64] = X @ Wv_h, col 64 = 1.0 ----
                vaug = workp.tile([128, NT, Q + 1], bf16, tag="vaug", bufs=2)
                nc.vector.memset(vaug[:, :, Q:Q + 1], 1.0)
                for t in range(NT):
                    pv = psump.tile([128, Q], f32, tag="mm", bufs=1)
                    for nn in range(2):
                        nc.tensor.matmul(
                            pv[:],
                            xt[:, nn, t * 128:(t + 1) * 128],
                            wv_sb[:, hh, nn, :],
                            start=(nn == 0), stop=(nn == 1),
                        )
                    nc.vector.tensor_copy(vaug[:, t, 0:Q], pv[:])

                # ---- phase 2: attention per s-block ----
                for sb in range(NSB):
                    s_lo = sb * SB
                    pav = psump.tile([Q + 1, SB], f32, tag="av", bufs=1)
                    for j in range((NT + TB - 1) // TB):
                        t0 = j * TB
                        nb = min(TB, NT - t0)
                        pbig = psump.tile([128, TB, SB], f32, tag="big", bufs=2)
                        for kk in range(nb):
                            t = t0 + kk
                            for mc in range(2):
                                nc.tensor.matmul(
                                    pbig[:, kk, :],
                                    xt[:, mc, t * 128:(t + 1) * 128],
                                    xmt[:, mc, s_lo:s_lo + SB],
                                    start=(mc == 0), stop=(mc == 1),
                                )
                        et = workp.tile([128, TB, SB], bf16, tag="et", bufs=3)
                        nc.scalar.activation(et[:, :nb, :], pbig[:, :nb, :], Exp)
                        for kk in range(nb):
                            t = t0 + kk
                            nc.tensor.matmul(
                                pav[:],
                                vaug[:, t, :],
                                et[:, kk, :],
                                start=(t == 0), stop=(t == NT - 1),
                            )
                    # normalize: zht = pav[0:64] * broadcast(1 / pav[64])
                    rs = workp.tile([1, SB], f32, tag="rs", bufs=2)
                    nc.vector.reciprocal(rs[:], pav[Q:Q + 1, :])
                    prb = psump.tile([Q, SB], f32, tag="mm", bufs=1)
                    nc.tensor.matmul(prb[:], ones_sb[:], rs[:])
                    zraw = workp.tile([Q, SB], f32, tag="zraw", bufs=2)
                    nc.vector.tensor_copy(zraw[:], pav[0:Q, :])
                    nc.vector.tensor_mul(zht[:, hh, s_lo:s_lo + SB], zraw[:], prb[:])

              # ---- phase 3: out = sum_h ZhT_h^T @ W0_h ----
              for sc in range(NT):
                po = psump.tile([128, Z], f32, tag="mm", bufs=1)
                for hh in range(HEADS_PER_CORE):
                    nc.tensor.matmul(
                        po[:],
                        zht[:, hh, sc * 128:(sc + 1) * 128],
                        w0_sb[:, hh, :],
                        start=(hh == 0), stop=(hh == HEADS_PER_CORE - 1),
                    )
                ot = workp.tile([128, Z], f32, tag="ot", bufs=3)
                nc.vector.tensor_copy(ot[:], po[:])
                nc.sync.dma_start(out_d[sc * 128:(sc + 1) * 128, :], ot[:])

    nc.compile()
    return nc


_NC_CACHE = {}


def _get_program(reps=1):
    if reps not in _NC_CACHE:
        _NC_CACHE[reps] = _build_program(reps)
    return _NC_CACHE[reps]


def _make_in_maps(X, M, W_v, W0):
    w0h = np.ascontiguousarray(W0.reshape(H, Q, Z))
    in_maps = []
    for c in range(NCORES):
        b = c // 4
        h0 = HEADS_PER_CORE * (c % 4)
        hs = [h0 + i for i in range(HEADS_PER_CORE)]
        in_maps.append({
            "xt": np.ascontiguousarray(X[b].T).astype(_bf16),
            "m": np.ascontiguousarray(M[hs] / float(Q)).astype(_bf16),
            "wv": np.ascontiguousarray(W_v[hs]).astype(_bf16),
            "w0": w0h[hs].astype(_bf16),
        })
    return in_maps


def run(X, M, W_v, W0, trace=False):
    from concourse.bass_utils import run_bass_kernel_spmd

    nc = _get_program()
    in_maps = _make_in_maps(X, M, W_v, W0)
    res = run_bass_kernel_spmd(nc, in_maps, list(range(NCORES)), trace=trace)
    out = np.zeros((B, S, Z), np.float32)
    for c in range(NCORES):
        out[c // 4] += np.asarray(res.results[c]["out"], dtype=np.float32)
    return out, res


def kernel(X, M, W_v, W0):
    out, _ = run(np.asarray(X), np.asarray(M), np.asarray(W_v), np.asarray(W0))
    return out
